# revision 6
# baseline (speedup 1.0000x reference)
"""Trainium2 Bass kernel for a DGCNN-style message-passing GNN.

Problem: B=2, N=512 nodes. Three EdgeConv layers with dense [N,N] edge MLPs
and masked max-aggregation, plus small pre/post MLPs.

Key algebraic restructuring (exact, up to matmul rounding):
  - EdgeConv first layer is linear in [x_i, x_j - x_i]:
        pre_ij = a_i + c_j,  a_i = x_i @ (W1a - W1b) + b1,  c_j = x_j @ W1b
    so the only per-edge work is relu(a_i + c_j) @ W2 and the masked max.
  - Masked max:  max_{j in N(i)} relu(z_ij + b2)
        = relu(b2 + max_j (z_ij + (A_ij - 1) * 32768))
    (relu is monotone; masked entries get -32768 which never wins).
    The (A-1)*32768 mask term is folded into the z matmul via PSUM
    accumulation (a K=2 selector matmul), so aggregation is a single
    PSUM reduce_max on the vector engine.

Sharding: batch b -> core group (4b..4b+3); destination rows i sharded 128
per core (global node order everywhere). AllGather (4-core groups)
exchanges node features between EdgeConv layers.

Per-core "rr layout": the 128 owned rows are processed as 64 groups
pairing local rows (g, 64+g); SBUF partitions hold [feat(i0); feat(i1)].
"""

import os
import sys

sys.path.insert(0, "/opt/trn_rl_repo")

import numpy as np

B = 2
N = 512
N_CORES = 8
RANKS = 4  # cores per batch
R = N // RANKS  # rows per core = 128
G = R // 2  # groups per core = 64
BIG = 32768.0  # power of two: exact in f32/f32r/bf16

_BUILT = None
LAST_EXEC_NS = None
LAST_RESULTS = None


def _build():
    import concourse.bass as bass
    import concourse.tile as tile
    from concourse import bacc, mybir

    f32 = mybir.dt.float32
    bf16 = mybir.dt.bfloat16
    AF = mybir.ActivationFunctionType
    OP = mybir.AluOpType
    AX = mybir.AxisListType

    nc = bacc.Bacc("TRN2", target_bir_lowering=False, debug=False,
                   num_devices=N_CORES)

    def inp(name, shape):
        return nc.dram_tensor(name, list(shape), f32, kind="ExternalInput")

    # ---- external inputs (per-core values supplied host-side) ----
    coordsT = inp("coordsT", [3, N])
    nfcs = inp("nfcs", [64, N])          # [node_features.T ; center_scores.T]
    coordsT_own = inp("coordsT_own", [3, R])
    nfcs_own = inp("nfcs_own", [64, R])
    a_own = inp("a_own", [R, N])         # adjacency rows owned by this core
    w_h1 = inp("w_h1", [3, 64])
    bh1c = inp("bh1c", [64, 1])
    w_h2 = inp("w_h2", [64, 64])
    bh2c = inp("bh2c", [64, 1])
    # per-EdgeConv weights (K_in = 128, 64, 64)
    ec_w = []
    for k, kin in zip((1, 2, 3), (128, 64, 64)):
        d = dict(
            wb2=inp(f"ec{k}_wb2", [kin, 128]),
            b1_2c=inp(f"ec{k}_b1_2c", [128, 1]),
            w2blk=inp(f"ec{k}_w2blk", [128, 128]),
            b2_2c=inp(f"ec{k}_b2_2c", [128, 1]),
            kin=kin,
        )
        if k == 1:
            d["wd_lo"] = inp("ec1_wd_lo", [kin, 128])
            d["wd_hi"] = inp("ec1_wd_hi", [kin, 128])
        else:
            d["wdblk"] = inp(f"ec{k}_wdblk", [128, 128])
        ec_w.append(d)
    sel2 = inp("sel2", [2, 128])
    wh6_lo = inp("wh6_lo", [128, 128])
    wh6_hi = inp("wh6_hi", [128, 128])
    bh6c = inp("bh6c", [128, 1])
    wout = inp("wout", [128, 3])
    boutc = inp("boutc", [3, 1])

    out_d = nc.dram_tensor("out", [3, R], f32, kind="ExternalOutput")

    # collective bounce buffers (node-feature exchange between EdgeConvs)
    bounce = [nc.dram_tensor(f"bounce{k}", [64, R], f32) for k in (0, 1)]
    gath = [nc.dram_tensor(f"gath{k}", [64 * RANKS, R], f32) for k in (0, 1)]
    groups = [[0, 1, 2, 3], [4, 5, 6, 7]]

    with tile.TileContext(nc) as tc, \
            tc.tile_pool(name="persist", bufs=1) as persist, \
            tc.tile_pool(name="weights", bufs=1) as wpool, \
            tc.tile_pool(name="mtiles", bufs=4) as mpool, \
            tc.tile_pool(name="small", bufs=2) as spool, \
            tc.tile_pool(name="psum_c2", bufs=1, space="PSUM") as pc2, \
            tc.tile_pool(name="psum_z", bufs=2, space="PSUM") as pz, \
            tc.tile_pool(name="psum_s", bufs=2, space="PSUM") as ps:

        def load(dram, shape, dt=f32):
            t = wpool.tile(list(shape), f32, tag=dram.name)
            nc.sync.dma_start(t[:], dram[:])
            if dt is not f32:
                tc2 = wpool.tile(list(shape), dt, tag=dram.name + "_c")
                nc.vector.tensor_copy(tc2[:], t[:])
                return tc2
            return t

        # ---- load weights/constants into SBUF ----
        s_wh1 = load(w_h1, (3, 64))
        s_bh1 = load(bh1c, (64, 1))
        s_wh2 = load(w_h2, (64, 64))
        s_bh2 = load(bh2c, (64, 1))
        s_ec = []
        for k in range(3):
            w = ec_w[k]
            kin = w["kin"]
            d = dict(
                wb2=load(w["wb2"], (kin, 128)),
                b1_2c=load(w["b1_2c"], (128, 1)),
                w2blk=load(w["w2blk"], (128, 128), dt=bf16),
                b2_2c=load(w["b2_2c"], (128, 1)),
                kin=kin,
            )
            if k == 0:
                d["wd_lo"] = load(w["wd_lo"], (kin, 128))
                d["wd_hi"] = load(w["wd_hi"], (kin, 128))
            else:
                d["wdblk"] = load(w["wdblk"], (128, 128))
            s_ec.append(d)
        s_sel2 = load(sel2, (2, 128), dt=bf16)
        s_wh6lo = load(wh6_lo, (128, 128))
        s_wh6hi = load(wh6_hi, (128, 128))
        s_bh6 = load(bh6c, (128, 1))
        s_wout = load(wout, (128, 3))
        s_bout = load(boutc, (3, 1))

        # ---- adjacency -> additive mask, pair-layout ----
        # mrows[p, j] = (A[p, j] - 1) * BIG   (0 for edges, -BIG otherwise)
        s_arows = persist.tile([R, N], f32)
        nc.sync.dma_start(s_arows[:], a_own[:])
        s_mrows = persist.tile([R, N], f32)
        nc.vector.tensor_scalar(s_mrows[:], s_arows[:], -1.0, BIG,
                                op0=OP.add, op1=OP.mult)
        # mpairs[k, g, j] = mrows[64*k + g, j]  (2 partitions, used as K=2 rhs)
        s_mrows_bf = persist.tile([R, N], bf16)
        nc.vector.tensor_copy(s_mrows_bf[:], s_mrows[:])
        s_mpairs = persist.tile([2, G, N], bf16)
        nc.sync.dma_start(s_mpairs[0:1, :, :], s_mrows_bf[0:G, :])
        nc.sync.dma_start(s_mpairs[1:2, :, :], s_mrows_bf[G:R, :])

        # ---- stage 1: x0T[128, N] = [relu MLP(coords) ; nf ; cs] ----
        s_coordsT = spool.tile([3, N], f32, tag="coordsT")
        nc.sync.dma_start(s_coordsT[:], coordsT[:])
        x0T = persist.tile([128, N], f32)
        nc.sync.dma_start(x0T[64:128, :], nfcs[:])
        p_h1 = ps.tile([64, N], f32, tag="sp")
        nc.tensor.matmul(p_h1[:], s_wh1[:], s_coordsT[:], start=True, stop=True)
        s_h1 = spool.tile([64, N], f32, tag="h1T")
        nc.scalar.activation(s_h1[:], p_h1[:], AF.Relu, bias=s_bh1[:])
        p_h2 = ps.tile([64, N], f32, tag="sp")
        nc.tensor.matmul(p_h2[:], s_wh2[:], s_h1[:], start=True, stop=True)
        nc.scalar.activation(x0T[0:64, :], p_h2[:], AF.Relu, bias=s_bh2[:])

        # same for the owned 128 columns only (feeds EC1's a-matmul)
        s_coordsTo = spool.tile([3, R], f32, tag="coordsTo")
        nc.sync.dma_start(s_coordsTo[:], coordsT_own[:])
        x0To = persist.tile([128, R], f32)
        nc.sync.dma_start(x0To[64:128, :], nfcs_own[:])
        p_h1o = ps.tile([64, R], f32, tag="sp")
        nc.tensor.matmul(p_h1o[:], s_wh1[:], s_coordsTo[:],
                         start=True, stop=True)
        s_h1o = spool.tile([64, R], f32, tag="h1To")
        nc.scalar.activation(s_h1o[:], p_h1o[:], AF.Relu, bias=s_bh1[:])
        p_h2o = ps.tile([64, R], f32, tag="sp")
        nc.tensor.matmul(p_h2o[:], s_wh2[:], s_h1o[:], start=True, stop=True)
        nc.scalar.activation(x0To[0:64, :], p_h2o[:], AF.Relu, bias=s_bh2[:])

        def edgeconv(xT, w, a_mms, rrprev=None):
            """One EdgeConv. xT: [kin, N] sbuf (feature-major, all nodes).
            a_mms: callable emitting the accumulation matmuls for the
            a-colmat psum tile. Returns rr [128, G] (rr layout)."""
            # a_colmat[p, g]: p<64 -> a[f, i=g], p>=64 -> a[f, i=64+g]
            p_a = ps.tile([128, G], f32, tag="sp")
            a_mms(p_a)
            a_cm = spool.tile([128, G], f32, tag="a_cm")
            nc.scalar.activation(a_cm[:], p_a[:], AF.Identity,
                                 bias=w["b1_2c"][:])
            # c2[p, j] = c[p % 64, j] (duplicated via column-tiled wb2)
            p_c2 = pc2.tile([128, N], f32)
            nc.tensor.matmul(p_c2[:], w["wb2"][:], xT[:],
                             start=True, stop=True)

            zmax = spool.tile([128, G], f32, tag="zmax")
            for gp in range(G // 2):
                zq = pz.tile([128, 2, N], f32, tag="zq")
                for q in range(2):
                    g = 2 * gp + q
                    m = mpool.tile([128, N], bf16, tag="m")
                    nc.scalar.activation(m[:], p_c2[:], AF.Relu,
                                         bias=a_cm[:, g:g + 1])
                    # mask first (independent of m), then z accumulates
                    nc.tensor.matmul(zq[:, q, :], s_sel2[:],
                                     s_mpairs[:, g, :],
                                     start=True, stop=False)
                    nc.tensor.matmul(zq[:, q, :], w["w2blk"][:], m[:],
                                     start=False, stop=True)
                nc.vector.tensor_reduce(zmax[:, 2 * gp:2 * gp + 2], zq[:],
                                        axis=AX.X, op=OP.max)
            rr = spool.tile([128, G], f32, tag="rr")
            nc.vector.tensor_scalar(rr[:], zmax[:], w["b2_2c"][:], 0.0,
                                    op0=OP.add, op1=OP.max)
            if rrprev is not None:
                nc.vector.tensor_tensor(rr[:], rr[:], rrprev[:], op=OP.add)
            return rr

        def exchange(rr, k):
            """AllGather rr (own rows, rr layout) -> xT [64, N] all nodes,
            global node order."""
            nc.sync.dma_start(bounce[k][:, 0:G], rr[0:64, :])
            nc.sync.dma_start(bounce[k][:, G:R], rr[64:128, :])
            nc.gpsimd.collective_compute(
                "AllGather", mybir.AluOpType.bypass, replica_groups=groups,
                ins=[bounce[k][:]], outs=[gath[k][:]])
            xT = persist.tile([64, N], f32, tag=f"xT{k}")
            nc.sync.dma_start(
                xT[:].rearrange("f (r i) -> f r i", r=RANKS),
                gath[k].ap().rearrange("(r f) i -> f r i", f=64))
            return xT

        def a_mms_ec1(p_a):
            nc.tensor.matmul(p_a[:], s_ec[0]["wd_lo"][:], x0To[:, 0:G],
                             start=True, stop=False)
            nc.tensor.matmul(p_a[:], s_ec[0]["wd_hi"][:], x0To[:, G:R],
                             start=False, stop=True)

        rr1 = edgeconv(x0T, s_ec[0], a_mms_ec1)
        g1 = exchange(rr1, 0)

        def a_mms_ec2(p_a):
            nc.tensor.matmul(p_a[:], s_ec[1]["wdblk"][:], rr1[:],
                             start=True, stop=True)

        rr2 = edgeconv(g1, s_ec[1], a_mms_ec2)
        g2 = exchange(rr2, 1)
        x3T = persist.tile([64, N], f32)
        nc.vector.tensor_tensor(x3T[:], g2[:], g1[:], op=OP.add)
        rr12 = spool.tile([128, G], f32, tag="rr12")
        nc.vector.tensor_tensor(rr12[:], rr2[:], rr1[:], op=OP.add)

        def a_mms_ec3(p_a):
            nc.tensor.matmul(p_a[:], s_ec[2]["wdblk"][:], rr12[:],
                             start=True, stop=True)

        rr3 = edgeconv(x3T, s_ec[2], a_mms_ec3, rrprev=rr2)

        # ---- final MLPs on rr3 [128, G] (2 stacked halves) ----
        for h, wh6 in ((0, s_wh6lo), (1, s_wh6hi)):
            p_h6 = ps.tile([128, G], f32, tag="sp")
            nc.tensor.matmul(p_h6[:], wh6[:], rr3[:], start=True, stop=True)
            s_h6 = spool.tile([128, G], f32, tag="h6")
            nc.scalar.activation(s_h6[:], p_h6[:], AF.Relu, bias=s_bh6[:])
            p_o = ps.tile([3, G], f32, tag="sp")
            nc.tensor.matmul(p_o[:], s_wout[:], s_h6[:], start=True, stop=True)
            s_o = spool.tile([3, G], f32, tag="so")
            nc.scalar.activation(s_o[:], p_o[:], AF.Relu, bias=s_bout[:])
            nc.sync.dma_start(out_d[:, h * G:(h + 1) * G], s_o[:])

    nc.compile()
    return nc


def _prep_inputs(coordinates, adjacency, node_features, center_scores,
                 W_h1, b_h1, W_h2, b_h2,
                 ec1_W1, ec1_b1, ec1_W2, ec1_b2,
                 ec2_W1, ec2_b1, ec2_W2, ec2_b2,
                 ec3_W1, ec3_b1, ec3_W2, ec3_b2,
                 W_h6, b_h6, W_out, b_out):
    """Build the 8 per-core input maps (numpy only, cheap)."""
    f = np.float32

    def col(v):
        return np.ascontiguousarray(np.asarray(v, f).reshape(-1, 1))

    shared = {
        "w_h1": np.ascontiguousarray(np.asarray(W_h1, f)),
        "bh1c": col(b_h1),
        "w_h2": np.ascontiguousarray(np.asarray(W_h2, f)),
        "bh2c": col(b_h2),
        "sel2": np.concatenate(
            [np.concatenate([np.ones((1, 64), f), np.zeros((1, 64), f)], 1),
             np.concatenate([np.zeros((1, 64), f), np.ones((1, 64), f)], 1)],
            0),
        "wh6_lo": np.concatenate(
            [np.asarray(W_h6, f), np.zeros((64, 128), f)], 0),
        "wh6_hi": np.concatenate(
            [np.zeros((64, 128), f), np.asarray(W_h6, f)], 0),
        "bh6c": col(b_h6),
        "wout": np.ascontiguousarray(np.asarray(W_out, f)),
        "boutc": col(b_out),
    }
    for k, (W1, b1, W2, b2) in enumerate(
            [(ec1_W1, ec1_b1, ec1_W2, ec1_b2),
             (ec2_W1, ec2_b1, ec2_W2, ec2_b2),
             (ec3_W1, ec3_b1, ec3_W2, ec3_b2)], start=1):
        W1 = np.asarray(W1, f)
        fin = W1.shape[0] // 2
        Wb = W1[fin:, :]
        Wd = W1[:fin, :] - Wb
        z = np.zeros_like(Wd)
        W2 = np.asarray(W2, f)
        z2 = np.zeros_like(W2)
        shared[f"ec{k}_wb2"] = np.concatenate([Wb, Wb], 1)
        if k == 1:
            shared["ec1_wd_lo"] = np.concatenate([Wd, z], 1)
            shared["ec1_wd_hi"] = np.concatenate([z, Wd], 1)
        else:
            shared[f"ec{k}_wdblk"] = np.block([[Wd, z], [z, Wd]]).astype(f)
        shared[f"ec{k}_b1_2c"] = col(np.concatenate([np.asarray(b1, f)] * 2))
        shared[f"ec{k}_w2blk"] = np.block([[W2, z2], [z2, W2]]).astype(f)
        shared[f"ec{k}_b2_2c"] = col(np.concatenate([np.asarray(b2, f)] * 2))
    shared = {k: np.ascontiguousarray(v) for k, v in shared.items()}

    coordinates = np.asarray(coordinates, f)
    adjacency = np.asarray(adjacency, f)
    node_features = np.asarray(node_features, f)
    center_scores = np.asarray(center_scores, f)

    in_maps = []
    for core in range(N_CORES):
        b, r = core // RANKS, core % RANKS
        own = slice(r * R, (r + 1) * R)
        m = dict(shared)
        m["coordsT"] = np.ascontiguousarray(coordinates[b].T)
        m["nfcs"] = np.ascontiguousarray(np.concatenate(
            [node_features[b].T, center_scores[b].T], 0))
        m["coordsT_own"] = np.ascontiguousarray(coordinates[b, own].T)
        m["nfcs_own"] = np.ascontiguousarray(np.concatenate(
            [node_features[b, own].T, center_scores[b, own].T], 0))
        m["a_own"] = np.ascontiguousarray(adjacency[b, own, :])
        in_maps.append(m)
    return in_maps


def kernel(**inputs):
    global _BUILT, LAST_EXEC_NS, LAST_RESULTS
    from concourse.bass_utils import run_bass_kernel_spmd

    if _BUILT is None:
        _BUILT = _build()
    nc = _BUILT

    in_maps = _prep_inputs(**inputs)
    trace = os.environ.get("KERNEL_TRACE", "0") == "1"
    res = run_bass_kernel_spmd(nc, in_maps, list(range(N_CORES)),
                               trace=trace)
    LAST_EXEC_NS = res.exec_time_ns
    LAST_RESULTS = res
    out = np.empty((B, N, 3), np.float32)
    for core in range(N_CORES):
        b, r = core // RANKS, core % RANKS
        out[b, r * R:(r + 1) * R, :] = res.results[core]["out"].T
    return out
